# revision 12
# baseline (speedup 1.0000x reference)
"""CWTConvNet Trainium2 kernel.

The reference computes a 112-filter Morlet-wavelet SAME conv over length-2048
signals, then indexes the result with IMG_SELECT = linspace(0, 71, 224) cast
to int64 — i.e. only conv output positions 0..71 survive, each repeated 1-4
times. For those 72 positions only filter taps k in [209, 561) can touch
nonzero (non-pad) input, so the whole module reduces exactly to

    out72[f, s, l] = sum_{j=0}^{351} w2[f, j] * xe[s, j + l],   l in [0, 72)

with w2 = w_real[:, 0, 209:561] and xe = [71 zeros, x[s, 0:352], pad], then an
index-repeat expansion 72 -> 224 along the last axis.

Device kernel (per core, pure data parallel over 4 of 32 batches = 48
signals): the host supplies xe with groups of 6 signals interleaved
element-wise, so each im2col DMA descriptor carries 6 signals (864B contiguous
runs instead of 144B — the im2col is descriptor-rate-bound otherwise).
Per 128-tap contraction chunk a DMA builds the [128, 4*432] moving operand;
accumulating matmuls produce one PSUM bank per 6-signal group; scalar/vector
engines drain + de-interleave PSUM to SBUF and perform the monotone
run-length expansion to [112, ., 224]; DMAs scatter straight into the
[4, 12, 112, 224] output layout. Work is pipelined over two signal halves
(DMA granularity) and four quarters (store granularity) across both HWDGE
rings; dummy matmuls on a zeroed tile warm the PE clock gate during the
initial DMA wait.
"""

import numpy as np

import concourse.bacc as bacc
import concourse.bass as bass
import concourse.mybir as mybir
import concourse.tile as tile
from concourse.bass_utils import run_bass_kernel_spmd

# Problem constants (hardcoded; kernel.py must be self-contained).
B, C, L = 32, 12, 2048
F, K = 112, 561
NCORES = 8
BPC = B // NCORES          # batches per core
S = BPC * C                # signals per core (48)
NL = 72                    # conv output positions actually used
NI = 224                   # expanded output length
J = 352                    # taps that can touch non-pad input: k in [209, 561)
KOFF = 209                 # first needed tap
NCHUNK = 3                 # contraction chunks of 128 (352 -> 128,128,96)
XE_LEN = 456               # 71 zeros + 352 signal + tail zeros (>= 2*128+127+71+1)
XE_ZLEAD = 71

TI = 6                     # signals interleaved per im2col descriptor
NG = S // TI               # signal groups per core (8)
NHALF = 2                  # DMA-granularity halves
GPH = NG // NHALF          # groups per half (4)
NCOL_U = TI * NL           # matmul columns per group/PSUM bank (432)
NQ = 4                     # store-granularity quarters (2 groups each)

# Config: input dtype for the matmul operands. fp32 is exact; bf16 halves
# im2col DMA bytes and matmul passes at ~2e-3 relative error.
USE_BF16 = True
WARMUP_MM = 10             # dummy matmuls to lift the PE HAM clock gate
# True: device emits the full [4,12,112,224]; False: device emits the 72
# unique conv columns [112,48,72] and the host performs the IMG_SELECT
# repeat-gather while unsharding (3.3MB less HBM store traffic per core).
EXPAND_ON_DEVICE = True

SEL = np.linspace(0, 71, NI, dtype=np.int64)

_CACHE = {}


def _expansion_ops():
    """Decompose i -> SEL[i] into copy ops: ('seg3', t0, t1, i0) covers
    t in [t0,t1) each repeated 3x contiguously from dst column i0;
    ('rep', t, r, i0) covers one t repeated r times."""
    i0 = np.searchsorted(SEL, np.arange(NL), side="left")
    runs = np.bincount(SEL, minlength=NL)
    ops = []
    t = 0
    while t < NL:
        if runs[t] == 3:
            t1 = t
            while t1 < NL and runs[t1] == 3:
                t1 += 1
            ops.append(("seg3", t, t1, int(i0[t])))
            t = t1
        else:
            ops.append(("rep", t, int(runs[t]), int(i0[t])))
            t += 1
    return ops


def _build_nc():
    f32 = mybir.dt.float32
    dt_in = mybir.dt.bfloat16 if USE_BF16 else f32
    nc = bacc.Bacc("TRN2", target_bir_lowering=False, debug=False)

    # xg[g, t, k] = xe[6g + k, t]  (6-signal element interleave), flat [NG, 2736]
    xg_d = nc.declare_dram_parameter("xg", [NG, XE_LEN * TI], dt_in, isOutput=False)
    w_d = nc.declare_dram_parameter("w2t", [128, NCHUNK, F], dt_in, isOutput=False)
    if EXPAND_ON_DEVICE:
        y_d = nc.declare_dram_parameter("y", [BPC, C, F, NI], f32, isOutput=True)
    else:
        y_d = nc.declare_dram_parameter("y", [F, S, NL], f32, isOutput=True)

    exp_ops = _expansion_ops()

    with tile.TileContext(nc) as tc:
        with (
            tc.tile_pool(name="sbuf", bufs=1) as pool,
            tc.tile_pool(name="psum", bufs=1, space="PSUM") as psum_pool,
        ):
            w_t = pool.tile([128, NCHUNK, F], dt_in, tag="w", name="w")
            nc.sync.dma_start(out=w_t[:], in_=w_d.ap())

            psum_u = [
                psum_pool.tile([128, NCOL_U], f32, tag=f"ps{u}", name=f"ps{u}")
                for u in range(NG)
            ]

            # PE warm-up: matmuls on a zeroed tile lift the HAM clock gate
            # (1.2 -> 2.4 GHz) while the im2col DMAs are in flight.
            if WARMUP_MM:
                scratch = pool.tile([128, 512], dt_in, tag="warm", name="warm")
                nc.gpsimd.memset(scratch[:], 0.0)
                for _ in range(WARMUP_MM):
                    nc.tensor.matmul(
                        psum_u[0][:F, :],
                        scratch[:, :F],
                        scratch[:, :NCOL_U],
                        start=True,
                        stop=True,
                    )

            # im2col per (half, chunk): rhs[p, g, (l k)] = xg[4h+g, (128jc+p+l)*6 + k]
            rhs = {}
            ring = {0: nc.sync, 1: nc.scalar}
            for h in range(NHALF):
                for jc in range(NCHUNK):
                    r_t = pool.tile(
                        [128, GPH, NCOL_U], dt_in,
                        tag=f"rhs{h}_{jc}", name=f"rhs{h}_{jc}",
                    )
                    src = bass.AP(
                        tensor=xg_d,
                        offset=h * GPH * XE_LEN * TI + 128 * jc * TI,
                        ap=[[TI, 128], [XE_LEN * TI, GPH], [1, NCOL_U]],
                    )
                    ring[(h + jc) % 2].dma_start(out=r_t[:], in_=src)
                    rhs[(h, jc)] = r_t

            if EXPAND_ON_DEVICE:
                y_all = y_d.ap().rearrange("b c f i -> f (b c) i")
            else:
                y_all = y_d.ap()
            seg3_engines = [nc.vector, nc.gpsimd]
            for h in range(NHALF):
                o72 = pool.tile([128, S // NHALF, NL], f32,
                                tag=f"out72_{h}", name=f"out72_{h}")
                if EXPAND_ON_DEVICE:
                    o224 = pool.tile([128, S // NHALF, NI], f32,
                                     tag=f"out224_{h}", name=f"out224_{h}")
                for qq in range(NQ // NHALF):
                    units = [2 * qq, 2 * qq + 1]  # groups within this half
                    # Matmuls: one PSUM bank per 6-signal group.
                    for jc in range(NCHUNK):
                        r_flat = rhs[(h, jc)].rearrange("p g n -> p (g n)")
                        for uu in units:
                            u = h * GPH + uu
                            nc.tensor.matmul(
                                psum_u[u][:F, :],
                                w_t[:, jc, :],
                                r_flat[:, uu * NCOL_U : (uu + 1) * NCOL_U],
                                start=(jc == 0),
                                stop=(jc == NCHUNK - 1),
                            )
                    # Drain + de-interleave PSUM -> SBUF: cols (l, k) -> [s, l].
                    for uu in units:
                        u = h * GPH + uu
                        src = psum_u[u][:F].rearrange("p (l k) -> p l k", k=TI)
                        dst = o72[:F, uu * TI : (uu + 1) * TI, :].rearrange(
                            "p k l -> p l k"
                        )
                        if uu % 2 == 0:
                            nc.scalar.copy(dst, src)
                        else:
                            nc.vector.tensor_copy(out=dst, in_=src)

                    s0 = qq * 2 * TI
                    g0 = h * (S // NHALF) + s0
                    if not EXPAND_ON_DEVICE:
                        # Store unique conv columns; host does the repeat-gather.
                        ring[(h * 2 + qq) % 2].dma_start(
                            out=y_all[:, g0 : g0 + 2 * TI, :],
                            in_=o72[:F, s0 : s0 + 2 * TI, :],
                        )
                        continue

                    # Expansion 72 -> 224 for this quarter (12 signals).
                    o72q = o72[:F, s0 : s0 + 2 * TI, :]
                    o224q = o224[:F, s0 : s0 + 2 * TI, :]
                    for ki, op in enumerate(exp_ops):
                        if op[0] == "seg3":
                            _, t0, t1, i0 = op
                            tlen = t1 - t0
                            dst = o224q[:, :, i0 : i0 + 3 * tlen].rearrange(
                                "p s (t d) -> p s t d", d=3
                            )
                            src = (
                                o72q[:, :, t0:t1]
                                .unsqueeze(3)
                                .broadcast_to([F, 2 * TI, tlen, 3])
                            )
                            eng = seg3_engines[ki % 2]
                            eng.tensor_copy(out=dst, in_=src)
                        else:
                            _, t, r, i0 = op
                            dst = o224q[:, :, i0 : i0 + r]
                            src = o72q[:, :, t : t + 1].broadcast_to([F, 2 * TI, r])
                            nc.scalar.copy(dst, src)

                    # Store this quarter: [f, s, i] -> y[b, c, f, i].
                    ring[(h * 2 + qq) % 2].dma_start(
                        out=y_all[:, g0 : g0 + 2 * TI, :],
                        in_=o224[:F, s0 : s0 + 2 * TI, :],
                    )

    nc.compile()
    return nc


def _get_nc():
    if "nc" not in _CACHE:
        _CACHE["nc"] = _build_nc()
    return _CACHE["nc"]


def _prepare_in_maps(x, w_real):
    if USE_BF16:
        import ml_dtypes

        np_in = np.dtype(ml_dtypes.bfloat16)
    else:
        np_in = np.dtype(np.float32)
    x = np.ascontiguousarray(np.asarray(x), dtype=np.float32)
    w_real = np.asarray(w_real, dtype=np.float32)

    w2t = np.zeros((NCHUNK * 128, F), np.float32)
    w2t[:J] = w_real[:, 0, KOFF:K].T
    w2t_dev = np.ascontiguousarray(
        w2t.reshape(NCHUNK, 128, F).transpose(1, 0, 2)
    ).astype(np_in)

    in_maps = []
    for m in range(NCORES):
        xe = np.zeros((S, XE_LEN), np.float32)
        xe[:, XE_ZLEAD : XE_ZLEAD + J] = x[m * BPC : (m + 1) * BPC].reshape(
            S, L
        )[:, :J]
        # interleave: xg[g, t, k] = xe[6g + k, t]
        xg = np.ascontiguousarray(
            xe.reshape(NG, TI, XE_LEN).transpose(0, 2, 1)
        ).reshape(NG, XE_LEN * TI)
        in_maps.append({"xg": xg.astype(np_in), "w2t": w2t_dev})
    return in_maps


def _assemble(results):
    if EXPAND_ON_DEVICE:
        return np.concatenate([r["y"] for r in results], axis=0)
    # results: [F, S, NL] per core -> gather IMG_SELECT and unshard.
    y72 = np.stack([r["y"] for r in results])          # [8, F, S, NL]
    y = y72[..., SEL]                                  # [8, F, S, NI]
    y = y.transpose(0, 2, 1, 3).reshape(NCORES, BPC, C, F, NI)
    return np.ascontiguousarray(y.reshape(B, C, F, NI))


def kernel(x, w_real):
    nc = _get_nc()
    in_maps = _prepare_in_maps(x, w_real)
    res = run_bass_kernel_spmd(nc, in_maps, list(range(NCORES)))
    return _assemble(res.results)


# revision 14
# speedup vs baseline: 2.1006x; 2.1006x over previous
"""CWTConvNet Trainium2 kernel.

The reference computes a 112-filter Morlet-wavelet SAME conv over length-2048
signals, then indexes the result with IMG_SELECT = linspace(0, 71, 224) cast
to int64 — i.e. only conv output positions 0..71 survive, each repeated 1-4
times. For those 72 positions only filter taps k in [209, 561) can touch
nonzero (non-pad) input, so the whole module reduces exactly to

    out72[f, s, l] = sum_{j=0}^{351} w2[f, j] * xe[s, j + l],   l in [0, 72)

with w2 = w_real[:, 0, 209:561] and xe = [71 zeros, x[s, 0:352], pad], then an
index-repeat expansion 72 -> 224 along the last axis.

Device kernel (per core, pure data parallel over 4 of 32 batches = 48
signals): the host supplies xe with groups of 6 signals interleaved
element-wise, so each im2col DMA descriptor carries 6 signals (864B contiguous
runs instead of 144B — the im2col is descriptor-rate-bound otherwise).
Per 128-tap contraction chunk a DMA builds the [128, 4*432] moving operand;
accumulating matmuls produce one PSUM bank per 6-signal group; scalar/vector
engines drain + de-interleave PSUM to SBUF and perform the monotone
run-length expansion to [112, ., 224]; DMAs scatter straight into the
[4, 12, 112, 224] output layout. Work is pipelined over two signal halves
(DMA granularity) and four quarters (store granularity) across both HWDGE
rings; dummy matmuls on a zeroed tile warm the PE clock gate during the
initial DMA wait.
"""

import numpy as np

import concourse.bacc as bacc
import concourse.bass as bass
import concourse.mybir as mybir
import concourse.tile as tile
from concourse.bass_utils import run_bass_kernel_spmd

# Problem constants (hardcoded; kernel.py must be self-contained).
B, C, L = 32, 12, 2048
F, K = 112, 561
NCORES = 8
BPC = B // NCORES          # batches per core
S = BPC * C                # signals per core (48)
NL = 72                    # conv output positions actually used
NI = 224                   # expanded output length
J = 352                    # taps that can touch non-pad input: k in [209, 561)
KOFF = 209                 # first needed tap
NCHUNK = 3                 # contraction chunks of 128 (352 -> 128,128,96)
XE_LEN = 456               # 71 zeros + 352 signal + tail zeros (>= 2*128+127+71+1)
XE_ZLEAD = 71

TI = 6                     # signals interleaved per im2col descriptor
NG = S // TI               # signal groups per core (8)
NHALF = 2                  # DMA-granularity halves
GPH = NG // NHALF          # groups per half (4)
NCOL_U = TI * NL           # matmul columns per group/PSUM bank (432)
NQ = 4                     # store-granularity quarters (2 groups each)

# Config: input dtype for the matmul operands. fp32 is exact; bf16 halves
# im2col DMA bytes and matmul passes at ~2e-3 relative error.
USE_BF16 = True
WARMUP_MM = 10             # dummy matmuls to lift the PE HAM clock gate
# True: device emits the full [4,12,112,224]; False: device emits the 72
# unique conv columns [112,48,72] and the host performs the IMG_SELECT
# repeat-gather while unsharding (3.3MB less HBM store traffic per core).
EXPAND_ON_DEVICE = False

SEL = np.linspace(0, 71, NI, dtype=np.int64)

_CACHE = {}


def _expansion_ops():
    """Decompose i -> SEL[i] into copy ops: ('seg3', t0, t1, i0) covers
    t in [t0,t1) each repeated 3x contiguously from dst column i0;
    ('rep', t, r, i0) covers one t repeated r times."""
    i0 = np.searchsorted(SEL, np.arange(NL), side="left")
    runs = np.bincount(SEL, minlength=NL)
    ops = []
    t = 0
    while t < NL:
        if runs[t] == 3:
            t1 = t
            while t1 < NL and runs[t1] == 3:
                t1 += 1
            ops.append(("seg3", t, t1, int(i0[t])))
            t = t1
        else:
            ops.append(("rep", t, int(runs[t]), int(i0[t])))
            t += 1
    return ops


def _build_nc():
    f32 = mybir.dt.float32
    dt_in = mybir.dt.bfloat16 if USE_BF16 else f32
    nc = bacc.Bacc("TRN2", target_bir_lowering=False, debug=False)

    # xg[g, t, k] = xe[6g + k, t]  (6-signal element interleave), flat [NG, 2736]
    xg_d = nc.declare_dram_parameter("xg", [NG, XE_LEN * TI], dt_in, isOutput=False)
    w_d = nc.declare_dram_parameter("w2t", [128, NCHUNK, F], dt_in, isOutput=False)
    if EXPAND_ON_DEVICE:
        y_d = nc.declare_dram_parameter("y", [BPC, C, F, NI], f32, isOutput=True)
    else:
        y_d = nc.declare_dram_parameter("y", [F, S, NL], f32, isOutput=True)

    exp_ops = _expansion_ops()

    with tile.TileContext(nc) as tc:
        with (
            tc.tile_pool(name="sbuf", bufs=1) as pool,
            tc.tile_pool(name="psum", bufs=1, space="PSUM") as psum_pool,
        ):
            w_t = pool.tile([128, NCHUNK, F], dt_in, tag="w", name="w")
            nc.sync.dma_start(out=w_t[:], in_=w_d.ap())

            psum_u = [
                psum_pool.tile([128, NCOL_U], f32, tag=f"ps{u}", name=f"ps{u}")
                for u in range(NG)
            ]

            # PE warm-up: matmuls on a zeroed tile lift the HAM clock gate
            # (1.2 -> 2.4 GHz) while the im2col DMAs are in flight.
            if WARMUP_MM:
                scratch = pool.tile([128, 512], dt_in, tag="warm", name="warm")
                nc.gpsimd.memset(scratch[:], 0.0)
                for _ in range(WARMUP_MM):
                    nc.tensor.matmul(
                        psum_u[0][:F, :],
                        scratch[:, :F],
                        scratch[:, :NCOL_U],
                        start=True,
                        stop=True,
                    )

            # im2col per (half, chunk): rhs[p, g, (l k)] = xg[4h+g, (128jc+p+l)*6 + k]
            rhs = {}
            ring = {0: nc.sync, 1: nc.scalar}
            for h in range(NHALF):
                for jc in range(NCHUNK):
                    r_t = pool.tile(
                        [128, GPH, NCOL_U], dt_in,
                        tag=f"rhs{h}_{jc}", name=f"rhs{h}_{jc}",
                    )
                    src = bass.AP(
                        tensor=xg_d,
                        offset=h * GPH * XE_LEN * TI + 128 * jc * TI,
                        ap=[[TI, 128], [XE_LEN * TI, GPH], [1, NCOL_U]],
                    )
                    ring[(h + jc) % 2].dma_start(out=r_t[:], in_=src)
                    rhs[(h, jc)] = r_t

            if EXPAND_ON_DEVICE:
                y_all = y_d.ap().rearrange("b c f i -> f (b c) i")
            else:
                y_all = y_d.ap()
            seg3_engines = [nc.vector, nc.vector]
            for h in range(NHALF):
                o72 = pool.tile([128, S // NHALF, NL], f32,
                                tag=f"out72_{h}", name=f"out72_{h}")
                if EXPAND_ON_DEVICE:
                    o224 = pool.tile([128, S // NHALF, NI], f32,
                                     tag=f"out224_{h}", name=f"out224_{h}")
                for qq in range(NQ // NHALF):
                    units = [2 * qq, 2 * qq + 1]  # groups within this half
                    # Matmuls: one PSUM bank per 6-signal group.
                    for jc in range(NCHUNK):
                        r_flat = rhs[(h, jc)].rearrange("p g n -> p (g n)")
                        for uu in units:
                            u = h * GPH + uu
                            nc.tensor.matmul(
                                psum_u[u][:F, :],
                                w_t[:, jc, :],
                                r_flat[:, uu * NCOL_U : (uu + 1) * NCOL_U],
                                start=(jc == 0),
                                stop=(jc == NCHUNK - 1),
                            )
                    # Drain + de-interleave PSUM -> SBUF: cols (l, k) -> [s, l].
                    for uu in units:
                        u = h * GPH + uu
                        src = psum_u[u][:F].rearrange("p (l k) -> p l k", k=TI)
                        dst = o72[:F, uu * TI : (uu + 1) * TI, :].rearrange(
                            "p k l -> p l k"
                        )
                        if uu % 2 == 0:
                            nc.scalar.copy(dst, src)
                        else:
                            nc.vector.tensor_copy(out=dst, in_=src)

                    s0 = qq * 2 * TI
                    g0 = h * (S // NHALF) + s0
                    if not EXPAND_ON_DEVICE:
                        # Store unique conv columns; host does the repeat-gather.
                        ring[(h * 2 + qq) % 2].dma_start(
                            out=y_all[:, g0 : g0 + 2 * TI, :],
                            in_=o72[:F, s0 : s0 + 2 * TI, :],
                        )
                        continue

                    # Expansion 72 -> 224 for this quarter (12 signals).
                    o72q = o72[:F, s0 : s0 + 2 * TI, :]
                    o224q = o224[:F, s0 : s0 + 2 * TI, :]
                    for ki, op in enumerate(exp_ops):
                        if op[0] == "seg3":
                            _, t0, t1, i0 = op
                            tlen = t1 - t0
                            dst = o224q[:, :, i0 : i0 + 3 * tlen].rearrange(
                                "p s (t d) -> p s t d", d=3
                            )
                            src = (
                                o72q[:, :, t0:t1]
                                .unsqueeze(3)
                                .broadcast_to([F, 2 * TI, tlen, 3])
                            )
                            eng = seg3_engines[ki % 2]
                            eng.tensor_copy(out=dst, in_=src)
                        else:
                            _, t, r, i0 = op
                            dst = o224q[:, :, i0 : i0 + r]
                            src = o72q[:, :, t : t + 1].broadcast_to([F, 2 * TI, r])
                            nc.scalar.copy(dst, src)

                    # Store this quarter: [f, s, i] -> y[b, c, f, i].
                    ring[(h * 2 + qq) % 2].dma_start(
                        out=y_all[:, g0 : g0 + 2 * TI, :],
                        in_=o224[:F, s0 : s0 + 2 * TI, :],
                    )

    nc.compile()
    return nc


def _get_nc():
    if "nc" not in _CACHE:
        _CACHE["nc"] = _build_nc()
    return _CACHE["nc"]


def _prepare_in_maps(x, w_real):
    if USE_BF16:
        import ml_dtypes

        np_in = np.dtype(ml_dtypes.bfloat16)
    else:
        np_in = np.dtype(np.float32)
    x = np.ascontiguousarray(np.asarray(x), dtype=np.float32)
    w_real = np.asarray(w_real, dtype=np.float32)

    w2t = np.zeros((NCHUNK * 128, F), np.float32)
    w2t[:J] = w_real[:, 0, KOFF:K].T
    w2t_dev = np.ascontiguousarray(
        w2t.reshape(NCHUNK, 128, F).transpose(1, 0, 2)
    ).astype(np_in)

    in_maps = []
    for m in range(NCORES):
        xe = np.zeros((S, XE_LEN), np.float32)
        xe[:, XE_ZLEAD : XE_ZLEAD + J] = x[m * BPC : (m + 1) * BPC].reshape(
            S, L
        )[:, :J]
        # interleave: xg[g, t, k] = xe[6g + k, t]
        xg = np.ascontiguousarray(
            xe.reshape(NG, TI, XE_LEN).transpose(0, 2, 1)
        ).reshape(NG, XE_LEN * TI)
        in_maps.append({"xg": xg.astype(np_in), "w2t": w2t_dev})
    return in_maps


def _assemble(results):
    if EXPAND_ON_DEVICE:
        return np.concatenate([r["y"] for r in results], axis=0)
    # results: [F, S, NL] per core -> gather IMG_SELECT and unshard.
    y72 = np.stack([r["y"] for r in results])          # [8, F, S, NL]
    y = y72[..., SEL]                                  # [8, F, S, NI]
    y = y.transpose(0, 2, 1, 3).reshape(NCORES, BPC, C, F, NI)
    return np.ascontiguousarray(y.reshape(B, C, F, NI))


def kernel(x, w_real):
    nc = _get_nc()
    in_maps = _prepare_in_maps(x, w_real)
    res = run_bass_kernel_spmd(nc, in_maps, list(range(NCORES)))
    return _assemble(res.results)


# revision 19
# speedup vs baseline: 2.2265x; 1.0599x over previous
"""CWTConvNet Trainium2 kernel.

The reference computes a 112-filter Morlet-wavelet SAME conv over length-2048
signals, then indexes the result with IMG_SELECT = linspace(0, 71, 224) cast
to int64 — i.e. only conv output positions 0..71 survive, each repeated 1-4
times. For those 72 positions only filter taps k in [209, 561) can touch
nonzero (non-pad) input, so the whole module reduces exactly to

    out72[f, s, l] = sum_{j=0}^{351} w2[f, j] * xe[s, j + l],   l in [0, 72)

with w2 = w_real[:, 0, 209:561] and xe = [71 zeros, x[s, 0:352], pad], then an
index-repeat expansion 72 -> 224 along the last axis.

Device kernel (per core, pure data parallel over 4 of 32 batches = 48
signals): the host supplies xe with groups of 6 signals interleaved
element-wise, so each im2col DMA descriptor carries 6 signals (864B contiguous
runs instead of 144B — the im2col is descriptor-rate-bound otherwise).
Per 128-tap contraction chunk a DMA builds the [128, 4*432] moving operand;
accumulating matmuls produce one PSUM bank per 6-signal group; scalar/vector
engines drain + de-interleave PSUM to SBUF and perform the monotone
run-length expansion to [112, ., 224]; DMAs scatter straight into the
[4, 12, 112, 224] output layout. Work is pipelined over two signal halves
(DMA granularity) and four quarters (store granularity) across both HWDGE
rings; dummy matmuls on a zeroed tile warm the PE clock gate during the
initial DMA wait.
"""

import numpy as np

import concourse.bacc as bacc
import concourse.bass as bass
import concourse.mybir as mybir
import concourse.tile as tile
from concourse.bass_utils import run_bass_kernel_spmd

# Problem constants (hardcoded; kernel.py must be self-contained).
B, C, L = 32, 12, 2048
F, K = 112, 561
NCORES = 8
BPC = B // NCORES          # batches per core
S = BPC * C                # signals per core (48)
NL = 72                    # conv output positions actually used
NI = 224                   # expanded output length
J = 352                    # taps that can touch non-pad input: k in [209, 561)
KOFF = 209                 # first needed tap
NCHUNK = 3                 # contraction chunks of 128 (352 -> 128,128,96)
XE_LEN = 456               # 71 zeros + 352 signal + tail zeros (>= 2*128+127+71+1)
XE_ZLEAD = 71

TI = 6                     # signals interleaved per im2col descriptor
NG = S // TI               # signal groups per core (8)
NHALF = 2                  # DMA-granularity halves
GPH = NG // NHALF          # groups per half (4)
NCOL_U = TI * NL           # matmul columns per group/PSUM bank (432)
NQ = 4                     # store-granularity quarters (2 groups each)

# Config: input dtype for the matmul operands. fp32 is exact; bf16 halves
# im2col DMA bytes and matmul passes at ~2e-3 relative error.
USE_BF16 = True
WARMUP_MM = 0              # dummy matmuls to lift the PE HAM clock gate
# True: device emits the full [4,12,112,224]; False: device emits the 72
# unique conv columns [112,48,72] and the host performs the IMG_SELECT
# repeat-gather while unsharding (3.3MB less HBM store traffic per core).
EXPAND_ON_DEVICE = False

SEL = np.linspace(0, 71, NI, dtype=np.int64)

_CACHE = {}


def _expansion_ops():
    """Decompose i -> SEL[i] into copy ops: ('seg3', t0, t1, i0) covers
    t in [t0,t1) each repeated 3x contiguously from dst column i0;
    ('rep', t, r, i0) covers one t repeated r times."""
    i0 = np.searchsorted(SEL, np.arange(NL), side="left")
    runs = np.bincount(SEL, minlength=NL)
    ops = []
    t = 0
    while t < NL:
        if runs[t] == 3:
            t1 = t
            while t1 < NL and runs[t1] == 3:
                t1 += 1
            ops.append(("seg3", t, t1, int(i0[t])))
            t = t1
        else:
            ops.append(("rep", t, int(runs[t]), int(i0[t])))
            t += 1
    return ops


def _build_nc():
    f32 = mybir.dt.float32
    dt_in = mybir.dt.bfloat16 if USE_BF16 else f32
    nc = bacc.Bacc("TRN2", target_bir_lowering=False, debug=False)

    # xg[g, t, k] = xe[6g + k, t]  (6-signal element interleave), flat [NG, 2736]
    xg_d = nc.declare_dram_parameter("xg", [NG, XE_LEN * TI], dt_in, isOutput=False)
    w_d = nc.declare_dram_parameter("w2t", [128, NCHUNK, F], dt_in, isOutput=False)
    if EXPAND_ON_DEVICE:
        y_d = nc.declare_dram_parameter("y", [BPC, C, F, NI], f32, isOutput=True)
    else:
        y_d = nc.declare_dram_parameter("y", [F, S, NL], f32, isOutput=True)

    exp_ops = _expansion_ops()

    with tile.TileContext(nc) as tc:
        with (
            tc.tile_pool(name="sbuf", bufs=1) as pool,
            tc.tile_pool(name="psum", bufs=1, space="PSUM") as psum_pool,
        ):
            w_t = pool.tile([128, NCHUNK, F], dt_in, tag="w", name="w")
            nc.scalar.dma_start(out=w_t[:], in_=w_d.ap())

            psum_u = [
                psum_pool.tile([128, NCOL_U], f32, tag=f"ps{u}", name=f"ps{u}")
                for u in range(NG)
            ]

            # PE warm-up: matmuls on a zeroed tile lift the HAM clock gate
            # (1.2 -> 2.4 GHz) while the im2col DMAs are in flight.
            if WARMUP_MM:
                scratch = pool.tile([128, 512], dt_in, tag="warm", name="warm")
                nc.gpsimd.memset(scratch[:], 0.0)
                for _ in range(WARMUP_MM):
                    nc.tensor.matmul(
                        psum_u[0][:F, :],
                        scratch[:, :F],
                        scratch[:, :NCOL_U],
                        start=True,
                        stop=True,
                    )

            # im2col per (half, chunk): rhs[p, g, (l k)] = xg[4h+g, (128jc+p+l)*6 + k]
            # All on the sync HWDGE ring: same-ring DMAs complete FIFO, so the
            # first chunk's data lands ~6x sooner than with concurrent rows,
            # letting matmuls start while later chunks stream.
            rhs = {}
            for h in range(NHALF):
                for jc in range(NCHUNK):
                    r_t = pool.tile(
                        [128, GPH, NCOL_U], dt_in,
                        tag=f"rhs{h}_{jc}", name=f"rhs{h}_{jc}",
                    )
                    src = bass.AP(
                        tensor=xg_d,
                        offset=h * GPH * XE_LEN * TI + 128 * jc * TI,
                        ap=[[TI, 128], [XE_LEN * TI, GPH], [1, NCOL_U]],
                    )
                    nc.sync.dma_start(out=r_t[:], in_=src)
                    rhs[(h, jc)] = r_t

            if EXPAND_ON_DEVICE:
                y_all = y_d.ap().rearrange("b c f i -> f (b c) i")
            else:
                y_all = y_d.ap()
            seg3_engines = [nc.vector, nc.vector]
            for h in range(NHALF):
                o72 = pool.tile([128, S // NHALF, NL], f32,
                                tag=f"out72_{h}", name=f"out72_{h}")
                if EXPAND_ON_DEVICE:
                    o224 = pool.tile([128, S // NHALF, NI], f32,
                                     tag=f"out224_{h}", name=f"out224_{h}")
                # Matmuls: one PSUM bank per 6-signal group, jc-outer so each
                # contraction chunk is consumed as soon as its DMA lands.
                for jc in range(NCHUNK):
                    r_flat = rhs[(h, jc)].rearrange("p g n -> p (g n)")
                    for uu in range(GPH):
                        u = h * GPH + uu
                        nc.tensor.matmul(
                            psum_u[u][:F, :],
                            w_t[:, jc, :],
                            r_flat[:, uu * NCOL_U : (uu + 1) * NCOL_U],
                            start=(jc == 0),
                            stop=(jc == NCHUNK - 1),
                        )
                # Drain + de-interleave PSUM -> SBUF: cols (l, k) -> [s, l].
                for uu in range(GPH):
                    u = h * GPH + uu
                    src = psum_u[u][:F].rearrange("p (l k) -> p l k", k=TI)
                    dst = o72[:F, uu * TI : (uu + 1) * TI, :].rearrange(
                        "p k l -> p l k"
                    )
                    if uu % 2 == 0:
                        nc.scalar.copy(dst, src)
                    else:
                        nc.vector.tensor_copy(out=dst, in_=src)

                for qq in range(NQ // NHALF):
                    s0 = qq * 2 * TI
                    g0 = h * (S // NHALF) + s0
                    if not EXPAND_ON_DEVICE:
                        # Store unique conv columns; host does the repeat-gather.
                        nc.scalar.dma_start(
                            out=y_all[:, g0 : g0 + 2 * TI, :],
                            in_=o72[:F, s0 : s0 + 2 * TI, :],
                        )
                        continue

                    # Expansion 72 -> 224 for this quarter (12 signals).
                    o72q = o72[:F, s0 : s0 + 2 * TI, :]
                    o224q = o224[:F, s0 : s0 + 2 * TI, :]
                    for ki, op in enumerate(exp_ops):
                        if op[0] == "seg3":
                            _, t0, t1, i0 = op
                            tlen = t1 - t0
                            dst = o224q[:, :, i0 : i0 + 3 * tlen].rearrange(
                                "p s (t d) -> p s t d", d=3
                            )
                            src = (
                                o72q[:, :, t0:t1]
                                .unsqueeze(3)
                                .broadcast_to([F, 2 * TI, tlen, 3])
                            )
                            eng = seg3_engines[ki % 2]
                            eng.tensor_copy(out=dst, in_=src)
                        else:
                            _, t, r, i0 = op
                            dst = o224q[:, :, i0 : i0 + r]
                            src = o72q[:, :, t : t + 1].broadcast_to([F, 2 * TI, r])
                            nc.scalar.copy(dst, src)

                    # Store this quarter: [f, s, i] -> y[b, c, f, i].
                    nc.scalar.dma_start(
                        out=y_all[:, g0 : g0 + 2 * TI, :],
                        in_=o224[:F, s0 : s0 + 2 * TI, :],
                    )

    nc.compile()
    return nc


def _get_nc():
    if "nc" not in _CACHE:
        _CACHE["nc"] = _build_nc()
    return _CACHE["nc"]


def _prepare_in_maps(x, w_real):
    if USE_BF16:
        import ml_dtypes

        np_in = np.dtype(ml_dtypes.bfloat16)
    else:
        np_in = np.dtype(np.float32)
    x = np.ascontiguousarray(np.asarray(x), dtype=np.float32)
    w_real = np.asarray(w_real, dtype=np.float32)

    w2t = np.zeros((NCHUNK * 128, F), np.float32)
    w2t[:J] = w_real[:, 0, KOFF:K].T
    w2t_dev = np.ascontiguousarray(
        w2t.reshape(NCHUNK, 128, F).transpose(1, 0, 2)
    ).astype(np_in)

    in_maps = []
    for m in range(NCORES):
        xe = np.zeros((S, XE_LEN), np.float32)
        xe[:, XE_ZLEAD : XE_ZLEAD + J] = x[m * BPC : (m + 1) * BPC].reshape(
            S, L
        )[:, :J]
        # interleave: xg[g, t, k] = xe[6g + k, t]
        xg = np.ascontiguousarray(
            xe.reshape(NG, TI, XE_LEN).transpose(0, 2, 1)
        ).reshape(NG, XE_LEN * TI)
        in_maps.append({"xg": xg.astype(np_in), "w2t": w2t_dev})
    return in_maps


def _assemble(results):
    if EXPAND_ON_DEVICE:
        return np.concatenate([r["y"] for r in results], axis=0)
    # results: [F, S, NL] per core -> gather IMG_SELECT and unshard.
    y72 = np.stack([r["y"] for r in results])          # [8, F, S, NL]
    y = y72[..., SEL]                                  # [8, F, S, NI]
    y = y.transpose(0, 2, 1, 3).reshape(NCORES, BPC, C, F, NI)
    return np.ascontiguousarray(y.reshape(B, C, F, NI))


def kernel(x, w_real):
    nc = _get_nc()
    in_maps = _prepare_in_maps(x, w_real)
    res = run_bass_kernel_spmd(nc, in_maps, list(range(NCORES)))
    return _assemble(res.results)


# revision 20
# speedup vs baseline: 2.3351x; 1.0488x over previous
"""CWTConvNet Trainium2 kernel.

The reference computes a 112-filter Morlet-wavelet SAME conv over length-2048
signals, then indexes the result with IMG_SELECT = linspace(0, 71, 224) cast
to int64 — i.e. only conv output positions 0..71 survive, each repeated 1-4
times. For those 72 positions only filter taps k in [209, 561) can touch
nonzero (non-pad) input, so the whole module reduces exactly to

    out72[f, s, l] = sum_{j=0}^{351} w2[f, j] * xe[s, j + l],   l in [0, 72)

with w2 = w_real[:, 0, 209:561] and xe = [71 zeros, x[s, 0:352], pad], then an
index-repeat expansion 72 -> 224 along the last axis.

Device kernel (per core, pure data parallel over 4 of 32 batches = 48
signals): the host supplies xe with groups of 12 signals interleaved
element-wise, so each im2col DMA descriptor carries 12 signals (1728B
contiguous runs — the im2col is descriptor-limited otherwise). Each group of
12 signals is an independent pipeline chain: 3 im2col DMAs (one per 128-tap
contraction chunk, all on the sync HWDGE ring so completions are FIFO),
2x3 accumulating matmuls into 2 PSUM banks, 2 plain PSUM->SBUF drains, and
one store on the scalar ring. The store keeps the (l, k)-interleaved PSUM
column order; the host undoes the interleave, applies the IMG_SELECT
repeat-gather, and unshards — all in one numpy pass.
"""

import numpy as np

import concourse.bacc as bacc
import concourse.bass as bass
import concourse.mybir as mybir
import concourse.tile as tile
from concourse.bass_utils import run_bass_kernel_spmd

# Problem constants (hardcoded; kernel.py must be self-contained).
B, C, L = 32, 12, 2048
F, K = 112, 561
NCORES = 8
BPC = B // NCORES          # batches per core
S = BPC * C                # signals per core (48)
NL = 72                    # conv output positions actually used
NI = 224                   # expanded output length
J = 352                    # taps that can touch non-pad input: k in [209, 561)
KOFF = 209                 # first needed tap
NCHUNK = 3                 # contraction chunks of 128 (352 -> 128,128,96)
XE_LEN = 456               # 71 zeros + 352 signal + tail zeros (>= 2*128+127+71+1)
XE_ZLEAD = 71

TI = 12                    # signals interleaved per im2col descriptor
NG = S // TI               # signal groups / pipeline chains per core (4)
NCOL_G = TI * NL           # matmul columns per group (864)
NBANK = 2                  # PSUM banks per group (864 fp32 cols)
NCOL_B = NCOL_G // NBANK   # columns per bank / matmul (432)
LPB = NL // NBANK          # l-positions per bank (36)

# Config: input dtype for the matmul operands. fp32 is exact; bf16 halves
# im2col DMA bytes and matmul passes at ~2e-3 relative error.
USE_BF16 = True

SEL = np.linspace(0, 71, NI, dtype=np.int64)

_CACHE = {}


def _build_nc():
    f32 = mybir.dt.float32
    dt_in = mybir.dt.bfloat16 if USE_BF16 else f32
    nc = bacc.Bacc("TRN2", target_bir_lowering=False, debug=False)

    # xg[g, t, k] = xe[12g + k, t]  (12-signal element interleave)
    xg_d = nc.declare_dram_parameter("xg", [NG, XE_LEN * TI], dt_in, isOutput=False)
    w_d = nc.declare_dram_parameter("w2t", [128, NCHUNK, F], dt_in, isOutput=False)
    # y[f, g, (l k)] keeps the interleaved PSUM column order; host undoes it.
    y_d = nc.declare_dram_parameter("y", [F, NG, NCOL_G], f32, isOutput=True)

    with tile.TileContext(nc) as tc:
        with (
            tc.tile_pool(name="sbuf", bufs=1) as pool,
            tc.tile_pool(name="psum", bufs=1, space="PSUM") as psum_pool,
        ):
            w_t = pool.tile([128, NCHUNK, F], dt_in, tag="w", name="w")
            nc.scalar.dma_start(out=w_t[:], in_=w_d.ap())

            psum_u = [
                psum_pool.tile([128, NCOL_B], f32, tag=f"ps{u}", name=f"ps{u}")
                for u in range(NG * NBANK)
            ]

            # im2col: rhs[p, (l k)] = xg[g, (128jc + p + l)*12 + k].
            # All on the sync ring: same-ring DMAs complete FIFO, so group 0's
            # chunks land first and its chain starts while later groups stream.
            rhs = {}
            for g in range(NG):
                for jc in range(NCHUNK):
                    r_t = pool.tile(
                        [128, NCOL_G], dt_in,
                        tag=f"rhs{g}_{jc}", name=f"rhs{g}_{jc}",
                    )
                    src = bass.AP(
                        tensor=xg_d,
                        offset=g * XE_LEN * TI + 128 * jc * TI,
                        ap=[[TI, 128], [1, NCOL_G]],
                    )
                    nc.sync.dma_start(out=r_t[:], in_=src)
                    rhs[(g, jc)] = r_t

            for g in range(NG):
                for jc in range(NCHUNK):
                    for b in range(NBANK):
                        nc.tensor.matmul(
                            psum_u[g * NBANK + b][:F, :],
                            w_t[:, jc, :],
                            rhs[(g, jc)][:, b * NCOL_B : (b + 1) * NCOL_B],
                            start=(jc == 0),
                            stop=(jc == NCHUNK - 1),
                        )
                # Plain contiguous drains (no de-interleave — host handles it).
                o72 = pool.tile([128, NCOL_G], f32, tag=f"o72_{g}", name=f"o72_{g}")
                for b in range(NBANK):
                    dst = o72[:F, b * NCOL_B : (b + 1) * NCOL_B]
                    if (g + b) % 2 == 0:
                        nc.scalar.copy(dst, psum_u[g * NBANK + b][:F, :])
                    else:
                        nc.vector.tensor_copy(out=dst, in_=psum_u[g * NBANK + b][:F, :])
                nc.scalar.dma_start(out=y_d.ap()[:, g, :], in_=o72[:F, :])

    nc.compile()
    return nc


def _get_nc():
    if "nc" not in _CACHE:
        _CACHE["nc"] = _build_nc()
    return _CACHE["nc"]


def _prepare_in_maps(x, w_real):
    if USE_BF16:
        import ml_dtypes

        np_in = np.dtype(ml_dtypes.bfloat16)
    else:
        np_in = np.dtype(np.float32)
    x = np.ascontiguousarray(np.asarray(x), dtype=np.float32)
    w_real = np.asarray(w_real, dtype=np.float32)

    w2t = np.zeros((NCHUNK * 128, F), np.float32)
    w2t[:J] = w_real[:, 0, KOFF:K].T
    w2t_dev = np.ascontiguousarray(
        w2t.reshape(NCHUNK, 128, F).transpose(1, 0, 2)
    ).astype(np_in)

    in_maps = []
    for m in range(NCORES):
        xe = np.zeros((S, XE_LEN), np.float32)
        xe[:, XE_ZLEAD : XE_ZLEAD + J] = x[m * BPC : (m + 1) * BPC].reshape(
            S, L
        )[:, :J]
        # interleave: xg[g, t, k] = xe[12g + k, t]
        xg = np.ascontiguousarray(
            xe.reshape(NG, TI, XE_LEN).transpose(0, 2, 1)
        ).reshape(NG, XE_LEN * TI)
        in_maps.append({"xg": xg.astype(np_in), "w2t": w2t_dev})
    return in_maps


def _assemble(results):
    # Device output: y[f, g, (l k)] with bank-major l split:
    # y[f, g, 432b + 12*lo + k] = out72[f, 12g + k, 36b + lo].
    ydev = np.stack([r["y"] for r in results])          # [8, F, NG, NCOL_G]
    yv = ydev.reshape(NCORES, F, NG, NBANK, LPB, TI)
    y72 = yv.transpose(0, 2, 5, 1, 3, 4)                # [8, NG, TI, F, NBANK, LPB]
    y72 = y72.reshape(NCORES, S, F, NL)                 # s = 12g + k, l = 36b + lo
    y = y72[..., SEL]                                   # [8, S, F, NI]
    return np.ascontiguousarray(y.reshape(B, C, F, NI))


def kernel(x, w_real):
    nc = _get_nc()
    in_maps = _prepare_in_maps(x, w_real)
    res = run_bass_kernel_spmd(nc, in_maps, list(range(NCORES)))
    return _assemble(res.results)
